# revision 36
# baseline (speedup 1.0000x reference)
"""Trainium2 Bass kernel for the text-CNN problem (dense_cnn).

Model: h = emb[x].reshape(B,1,L); three 1-channel 1D convs (K=3,4,5, 100
filters each) + bias + ReLU + global max-pool; concat; FC -> [B, 10].

Key identity: max_i relu(conv_i + b) == relu(b + max_i conv_i), so the
device only needs the raw per-filter max of each conv over all positions.

Device mapping (per core, 8-way shard over the 900k position axis):
  - conv as matmul: stationary [36, 128] packs 4 filters x 32 positions
    (Toeplitz bands, m = f_local*32 + r, entry [r+k, m] = w[f, 0, k]);
    moving operand is a stride-32 im2col of the signal: rhs[t, n] =
    sig[32*n + t], t in [0,36). One matmul column -> 128 useful outputs.
  - per (group, batch) "pack": 4 PSUM tiles [128, 896/862] (2-bank
    slots, 4-deep rotation over all 8 banks), 2 matmuls each.
  - drain: ScalarE copies tiles T0/T2 to SBUF bf16; DVE runs two
    independent tensor_tensor_scan(max, max) ops, each consuming one PSUM
    element and one SBUF element per cycle; each scan broadcast-writes its
    state onto one acc cell (last write wins = pair max) -> one DMA of
    acc[128, 300]; host maxes the column pairs.
Host: embedding gather, im2col prep (bf16), stationaries, final max over
r/cores, ragged-tail positions, ReLU+bias, FC.
"""

import os
import numpy as np

import concourse.bass as bass
import concourse.bacc as bacc
import concourse.mybir as mybir
from concourse.tile import TileContext
from concourse import bass_utils

import ml_dtypes

BF16 = ml_dtypes.bfloat16

# ---- problem constants (hardcoded; kernel.py must be self-contained) ----
VOCAB = 35097
WORD_DIM = 300
MAX_SENT = 3000
L = WORD_DIM * MAX_SENT          # 900000
B = 2
N_FILT = 100
KS = (3, 4, 5)
N_CLASSES = 10

N_CORES = 8
S = 32                            # positions per matmul column
TROWS = 36                        # S + max(K) - 1
GF = 4                            # filters per group
N_GROUPS = 3 * N_FILT // GF       # 75
TWS = (896, 896, 862, 862)        # PSUM tile widths (2-bank slots); the
                                  # two scan pairs are size-matched
NCOL_B = sum(TWS)                 # 3516 columns per batch (= ceil(112500/32))
NCOL = 2 * NCOL_B                 # 7032 columns per core
P5 = L - 5 + 1                    # 899996 valid positions for K=5
CHUNK = 112500                    # positions per core (8*112500 >= P5)
CSTART_MAX = P5 - S               # 899964 max column start

ACC_COLS = N_GROUPS * 4           # 300: two accum cols per (group, batch)


def _build_bass(n_groups=N_GROUPS, in_dt=mybir.dt.bfloat16):
    """Build the SPMD Bass module (same program on all cores).

    Per (group, batch): 4 PSUM tiles widths TWS (T0..T3; 2-bank slots, 8
    banks total, 4-slot rotation). ScalarE copies T0->cb0, T2->cb2 (bf16);
    DVE runs two independent tensor_tensor_scan(max, max) ops -- each
    consumes one PSUM and one SBUF element per cycle; each scan broadcast-
    writes its state onto one acc cell (last write = that pair's max).
    """
    nc = bacc.Bacc("TRN2", target_bir_lowering=False, debug=False,
                   num_devices=N_CORES)
    ncol = NCOL
    rhs_d = nc.dram_tensor("rhs", [TROWS, ncol], in_dt, kind="ExternalInput")
    wts_d = nc.dram_tensor("wts", [TROWS, n_groups * 128], in_dt,
                           kind="ExternalInput")
    acc_d = nc.dram_tensor("acc", [128, n_groups * 4], mybir.dt.float32,
                           kind="ExternalOutput")

    bf16 = mybir.dt.bfloat16
    MAX = mybir.AluOpType.max

    with TileContext(nc) as tc:
        with tc.tile_pool(name="io", bufs=1) as io_pool, \
             tc.tile_pool(name="cb", bufs=4) as c_pool, \
             tc.tile_pool(name="ps", bufs=4, space="PSUM") as psum_pool:
            rhs = io_pool.tile([TROWS, ncol], in_dt)
            wts = io_pool.tile([TROWS, n_groups * 128], in_dt)
            acc = io_pool.tile([128, n_groups * 4], mybir.dt.float32)
            nc.sync.dma_start(rhs[:, :], rhs_d[:, :])
            nc.sync.dma_start(wts[:, :], wts_d[:, :])
            tc.strict_bb_all_engine_barrier()

            for g in range(n_groups):
                lhsT = wts[:, g * 128:(g + 1) * 128]
                for b in range(2):
                    col0 = b * NCOL_B           # rhs col base for this batch
                    c0 = g * 2 + b
                    tiles = []
                    toff = 0
                    for t, tw in enumerate(TWS):
                        ps = psum_pool.tile([128, tw], mybir.dt.float32,
                                            tag="ps")
                        for jo, jn in ((0, 512), (512, tw - 512)):
                            o = col0 + toff + jo
                            nc.tensor.matmul(
                                ps[:, jo:jo + jn], lhsT,
                                rhs[:, o:o + jn], start=True, stop=True)
                        tiles.append(ps)
                        toff += tw

                    for pair in range(2):
                        tw = TWS[2 * pair]
                        cb = c_pool.tile([128, tw], bf16, tag="cbuf")
                        nc.scalar.copy(cb[:, :], tiles[2 * pair][:, :])
                        # scan state broadcast-writes one cell; the last
                        # write is the running max of both streams
                        dst = acc[:, 2 * c0 + pair:2 * c0 + pair + 1]
                        init = -3.0e38
                        nc.vector.tensor_tensor_scan(
                            dst.broadcast_to([128, tw]),
                            tiles[2 * pair + 1][:, :], cb[:, :],
                            init, op0=MAX, op1=MAX)

            nc.sync.dma_start(acc_d[:, :], acc[:, :])
    nc.compile()
    return nc


# ---------------- host-side preparation ----------------

def _build_stationary(w1, w2, w3):
    """[TROWS, N_GROUPS*128]: group g covers filters 4g..4g+3 of its conv,
    column m = f_local*32 + r, entry [r+k, m] = w[f, 0, k]."""
    ws = np.zeros((TROWS, N_GROUPS * 128), np.float32)
    convs = [(np.asarray(w1, np.float32), 3),
             (np.asarray(w2, np.float32), 4),
             (np.asarray(w3, np.float32), 5)]
    g = 0
    for w, K in convs:
        for g_local in range(N_FILT // GF):
            for fl in range(GF):
                f = g_local * GF + fl
                for r in range(S):
                    ws[r:r + K, g * 128 + fl * S + r] = w[f, 0, :]
            g += 1
    return ws


def _column_starts(core):
    base = core * CHUNK
    starts = base + S * np.arange(NCOL_B)
    return np.minimum(starts, CSTART_MAX)


def _make_rhs(sig, core, dtype):
    """sig: [B, L] fp32 -> [TROWS, 2*NCOL_B] im2col for this core."""
    starts = _column_starts(core)
    cols = []
    for b in range(B):
        win = np.lib.stride_tricks.sliding_window_view(sig[b], TROWS)
        cols.append(win[starts].T)          # [TROWS, NCOL_B]
    return np.ascontiguousarray(np.concatenate(cols, axis=1)).astype(dtype)


_CACHE = {}


def _get_nc():
    if "nc" not in _CACHE:
        _CACHE["nc"] = _build_bass()
    return _CACHE["nc"]


def _device_acc(rhs_list, wts):
    """Run the bass kernel on 8 cores. rhs_list[i]: [TROWS, 2*NCOL_B].
    Returns list of acc arrays [128, ACC_COLS] fp32."""
    if os.environ.get("KERNEL_EMULATE"):
        out = []
        for rhs in rhs_list:
            acc = np.empty((128, ACC_COLS), np.float32)
            for g in range(N_GROUPS):
                pg = np.einsum("tm,tn->mn",
                               wts[:, g * 128:(g + 1) * 128].astype(np.float32),
                               rhs.astype(np.float32))  # [128, 2*NCOL_B]
                half = TWS[0] + TWS[1]
                for b in range(2):
                    seg = pg[:, b * NCOL_B:(b + 1) * NCOL_B]
                    acc[:, 4 * g + 2 * b] = seg[:, :half].max(axis=1)
                    acc[:, 4 * g + 2 * b + 1] = seg[:, half:].max(axis=1)
            out.append(acc)
        return out

    nc = _get_nc()
    in_maps = [{"rhs": rhs, "wts": wts} for rhs in rhs_list]
    res = bass_utils.run_bass_kernel_spmd(nc, in_maps,
                                          core_ids=list(range(N_CORES)))
    return [r["acc"] for r in res.results]


def kernel(x, emb, w1, b1, w2, b2, w3, b3, fc_w, fc_b):
    x = np.asarray(x)
    emb = np.asarray(emb, np.float32)
    sig = emb[x.reshape(-1)].reshape(B, L)          # [2, 900000] fp32

    wts = _build_stationary(w1, w2, w3).astype(BF16)
    rhs_list = [_make_rhs(sig, c, BF16) for c in range(N_CORES)]

    accs = _device_acc(rhs_list, wts)

    # acc[m, g*NBLK + blk]; blocks 0..6 batch0, 7..13 batch1
    # -> per-batch per-filter maxes
    conv_max = np.full((B, 3 * N_FILT), -np.inf, np.float32)
    for acc in accs:
        a = acc.reshape(128, N_GROUPS, 2, 2)
        for b in range(B):
            mb = a[:, :, b, :].max(axis=2)                  # [128, 75]
            # rows m = f_local*32 + r -> [GF, S, N_GROUPS] -> max over r
            mb = mb.reshape(GF, S, N_GROUPS).max(axis=1)           # [GF, 75]
            # filter id = group_base + (g_local*GF + f_local)
            mb = mb.T.reshape(3, N_FILT // GF, GF).reshape(3 * N_FILT)
            conv_max[b] = np.maximum(conv_max[b], mb)

    # ragged tail positions not covered on device (fp32 host math)
    w1a = np.asarray(w1, np.float32)
    w2a = np.asarray(w2, np.float32)
    for b in range(B):
        for p in (L - 3 + 1 - 1, L - 3 + 1 - 2):   # 899997, 899996 (K=3)
            if p > P5 - 1:
                v = sig[b, p:p + 3] @ w1a[:, 0, :].T
                conv_max[b, :N_FILT] = np.maximum(conv_max[b, :N_FILT], v)
        p = L - 4 + 1 - 1                           # 899996 (K=4)
        if p > P5 - 1:
            v = sig[b, p:p + 4] @ w2a[:, 0, :].T
            conv_max[b, N_FILT:2 * N_FILT] = \
                np.maximum(conv_max[b, N_FILT:2 * N_FILT], v)

    bias = np.concatenate([np.asarray(b1, np.float32),
                           np.asarray(b2, np.float32),
                           np.asarray(b3, np.float32)])
    feats = np.maximum(conv_max + bias[None, :], 0.0)
    out = feats @ np.asarray(fc_w, np.float32).T + np.asarray(fc_b, np.float32)
    return out.astype(np.float32)
